# revision 1
# baseline (speedup 1.0000x reference)
"""Trainium2 Bass kernel: sampled logistic-regression forward.

reference math (per data row i, sample s):
    mean_i = X[i] . w_mu
    var_i  = sum_d X[i,d]^2 * exp(w_log_var[d])
    out[i,s] = sigmoid( sqrt(var_i) * z[s] + mean_i )

Full shapes: X [500000, 64], w_mu [64], w_log_var [64], z [128]
Output: [500000, 128] fp32.

Sharding: data-parallel over 8 NeuronCores, 62500 rows each.

Per-core pipeline, blocks of 24 tiles x [125 rows, 64]:
  - DMA in X block
  - ACT: X2 = Square(X)            (sigmoid_and_others table set)
  - DVE: A = X * w_mu (bcast), reduce(A) -> mean; reduce(V) -> var
  - GPSIMD: V = X2 * exp(lv) (bcast, in place)
  - DVE: Newton rsqrt (bit-trick seed, 2 iters); std = var * y
  - stats split hi/lo to f32r precision (mantissa AND-mask) into a
    5-row-per-tile interleaved stats block [125, 5*24]
  - PE: transpose stats -> [120, 125]; full-rate float32r matmuls vs a
    constant block-diagonal Z2BIG [120, 24*128]:
      arg = mh*1 + ml*1 + sh*zh + sh*zl + sl*zh  ~= mean + std*z (~2^-24)
  - ACT: Sigmoid [125, 512] PSUM->SBUF (4-tile batches)
  - DMA out block

float32r matmuls stream 1 column/cycle (vs 1/4 for fp32); f32r
round-to-nearest keeps 12 mantissa bits, so the hi+lo pair recovers
~fp32 accuracy.
"""

from contextlib import ExitStack

import numpy as np

import concourse.bacc as bacc
import concourse.bass as bass
import concourse.tile as tile
from concourse import mybir
from concourse.bass_utils import run_bass_kernel_spmd

N_CORES = 8
D = 64
NS = 128
P = 125          # rows per tile (partition dim)
BLK_T = 24       # tiles per block (5*24 = 120 = K of the affine matmul)
SIG_T = 4        # tiles per sigmoid ACT op (4*128 = 512 = one PSUM bank)
KR = 5           # K-rows per tile: mh, ml, sh(*zh), sh(*zl), sl(*zh)

RSQRT_MAGIC = 0x5F3759DF
F32R_MASK = 0xFFFFF000   # keep 11 explicit mantissa bits (f32r-representable)
F32 = mybir.dt.float32
F32R = mybir.dt.float32r
U32 = mybir.dt.uint32


def build_program(rows: int, nrep: int = 1):
    """Build the single-core Bass/Tile program for `rows` rows (SPMD across cores).

    nrep > 1 repeats the whole streaming body (for timing: per-iteration
    device time = (t(nrep=k) - t(nrep=1)) / (k - 1)).
    """
    assert rows % P == 0
    ntiles = rows // P
    assert ntiles % SIG_T == 0

    nc = bacc.Bacc(
        "TRN2",
        target_bir_lowering=False,
        debug=False,
        num_devices=N_CORES,
    )

    x = nc.dram_tensor("x", [rows, D], F32, kind="ExternalInput")
    wmu_d = nc.dram_tensor("wmu", [P, D], F32, kind="ExternalInput")
    elv_d = nc.dram_tensor("elv", [P, D], F32, kind="ExternalInput")
    z2big = nc.dram_tensor(
        "z2big", [KR * BLK_T, BLK_T * NS], F32R, kind="ExternalInput"
    )
    ident = nc.dram_tensor("ident", [P, P], F32, kind="ExternalInput")
    out = nc.dram_tensor("out", [rows, NS], F32, kind="ExternalOutput")

    xr = x.rearrange("(t p) d -> p t d", p=P)        # [125, ntiles, 64]
    outr = out.rearrange("(t p) s -> p t s", p=P)    # [125, ntiles, 128]

    nblocks = (ntiles + BLK_T - 1) // BLK_T

    with tile.TileContext(nc) as tc, ExitStack() as ctx:
        singles = ctx.enter_context(tc.tile_pool(name="singles", bufs=1))
        xin = ctx.enter_context(tc.tile_pool(name="xin", bufs=4))
        sqp = ctx.enter_context(tc.tile_pool(name="sqp", bufs=3))
        amp = ctx.enter_context(tc.tile_pool(name="amp", bufs=3))
        statp = ctx.enter_context(tc.tile_pool(name="statp", bufs=3))
        smalls = ctx.enter_context(tc.tile_pool(name="smalls", bufs=4))
        s2p = ctx.enter_context(tc.tile_pool(name="s2p", bufs=3))
        outp = ctx.enter_context(tc.tile_pool(name="outp", bufs=3))
        pst_pool = ctx.enter_context(tc.tile_pool(name="pst", bufs=2, space="PSUM"))
        paff_pool = ctx.enter_context(tc.tile_pool(name="paff", bufs=4, space="PSUM"))

        # one-time loads; broadcast weights are landed on their consumer
        # engines via a copy (keeps per-instruction sync-wait fan-in low)
        wmu_stage = singles.tile([P, 1, D], F32)
        nc.sync.dma_start(out=wmu_stage, in_=wmu_d.rearrange("p (o d) -> p o d", d=D))
        wmu_sb = singles.tile([P, 1, D], F32)
        nc.vector.tensor_copy(wmu_sb, wmu_stage)
        elv_stage = singles.tile([P, 1, D], F32)
        nc.sync.dma_start(out=elv_stage, in_=elv_d.rearrange("p (o d) -> p o d", d=D))
        elv_sb = singles.tile([P, 1, D], F32)
        nc.gpsimd.tensor_copy(elv_sb, elv_stage)
        z2_sb = singles.tile([KR * BLK_T, BLK_T * NS], F32R)
        nc.sync.dma_start(out=z2_sb, in_=z2big[:, :])
        id_stage = singles.tile([P, P], F32)
        nc.sync.dma_start(out=id_stage, in_=ident[:, :])
        id_sb = singles.tile([P, P], F32)
        nc.vector.tensor_copy(id_sb, id_stage)
        magic_sb = singles.tile([P, BLK_T], U32)
        nc.vector.memset(magic_sb, RSQRT_MAGIC)
        one_sb = singles.tile([P, 1], U32)
        nc.vector.memset(one_sb, 1)
        mask_sb = singles.tile([P, 1], U32)
        nc.vector.memset(mask_sb, F32R_MASK)

        for _rep in range(nrep):
          for b in range(nblocks):
            t0 = b * BLK_T
            T = min(BLK_T, ntiles - t0)
            tb = KR * T

            xt = xin.tile([P, BLK_T, D], F32)
            nc.sync.dma_start(out=xt[:, :T, :], in_=xr[:, t0 : t0 + T, :])

            # X^2 on ACT (Square lives in the sigmoid table set)
            x2 = sqp.tile([P, BLK_T, D], F32)
            nc.scalar.activation(
                out=x2[:, :T, :], in_=xt[:, :T, :],
                func=mybir.ActivationFunctionType.Square,
            )
            # A = X * w_mu (broadcast along tile dim) on DVE
            at = amp.tile([P, BLK_T, D], F32)
            nc.vector.tensor_mul(
                at[:, :T, :], xt[:, :T, :], wmu_sb.to_broadcast([P, T, D])
            )
            # V = X^2 * exp(lv) in place on GPSIMD
            nc.gpsimd.tensor_mul(
                x2[:, :T, :], x2[:, :T, :], elv_sb.to_broadcast([P, T, D])
            )

            mean_t = smalls.tile([P, BLK_T], F32)
            nc.vector.tensor_reduce(
                out=mean_t[:, :T],
                in_=at[:, :T, :],
                axis=mybir.AxisListType.X,
                op=mybir.AluOpType.add,
            )
            var = smalls.tile([P, BLK_T], F32)
            nc.vector.tensor_reduce(
                out=var[:, :T],
                in_=x2[:, :T, :],
                axis=mybir.AxisListType.X,
                op=mybir.AluOpType.add,
            )

            # y = rsqrt(var) on DVE: seed 0x5f3759df - (bits >> 1), 2 NR iters
            vb = var[:, :T].bitcast(U32)
            yb = smalls.tile([P, BLK_T], U32)
            nc.vector.tensor_scalar(
                yb[:, :T], vb, one_sb[:, 0:1], None,
                op0=mybir.AluOpType.logical_shift_right,
            )
            nc.vector.scalar_tensor_tensor(
                out=yb[:, :T],
                in0=magic_sb[:, :T],
                scalar=0,
                in1=yb[:, :T],
                op0=mybir.AluOpType.bypass,
                op1=mybir.AluOpType.subtract,
            )
            y = yb.bitcast(F32)
            t2 = smalls.tile([P, BLK_T], F32)
            for _ in range(2):
                nc.vector.tensor_mul(t2[:, :T], y[:, :T], y[:, :T])
                nc.vector.tensor_mul(t2[:, :T], t2[:, :T], var[:, :T])
                nc.vector.tensor_scalar(
                    t2[:, :T], t2[:, :T], -0.5, 1.5,
                    op0=mybir.AluOpType.mult,
                    op1=mybir.AluOpType.add,
                )
                nc.vector.tensor_mul(y[:, :T], y[:, :T], t2[:, :T])
            std_t = smalls.tile([P, BLK_T], F32)
            nc.vector.tensor_mul(std_t[:, :T], var[:, :T], y[:, :T])

            # split mean/std into f32r-representable hi/lo rows:
            # statblk rows per tile: [mh, ml, sh, sh, sl]
            statblk = statp.tile([P, BLK_T, KR], F32)
            sb_u = statblk.bitcast(U32)
            rem = smalls.tile([P, BLK_T], F32)
            nc.vector.tensor_scalar(
                sb_u[:, :T, 0], mean_t[:, :T].bitcast(U32), mask_sb[:, 0:1], None,
                op0=mybir.AluOpType.bitwise_and,
            )
            nc.vector.tensor_sub(rem[:, :T], mean_t[:, :T], statblk[:, :T, 0])
            nc.vector.tensor_scalar(
                sb_u[:, :T, 1], rem[:, :T].bitcast(U32), mask_sb[:, 0:1], None,
                op0=mybir.AluOpType.bitwise_and,
            )
            nc.vector.tensor_scalar(
                sb_u[:, :T, 2], std_t[:, :T].bitcast(U32), mask_sb[:, 0:1], None,
                op0=mybir.AluOpType.bitwise_and,
            )
            nc.gpsimd.tensor_copy(statblk[:, :T, 3], statblk[:, :T, 2])
            nc.gpsimd.tensor_sub(rem[:, :T], std_t[:, :T], statblk[:, :T, 2])
            nc.vector.tensor_scalar(
                sb_u[:, :T, 4], rem[:, :T].bitcast(U32), mask_sb[:, 0:1], None,
                op0=mybir.AluOpType.bitwise_and,
            )

            # transpose stats block: [125, tb] -> [tb, 125] (PSUM), copy to SBUF
            pst = pst_pool.tile([KR * BLK_T, P], F32)
            nc.tensor.transpose(
                out=pst[:tb, :],
                in_=statblk.rearrange("p t k -> p (t k)")[:, :tb],
                identity=id_sb,
            )
            s2 = s2p.tile([KR * BLK_T, P], F32R)
            nc.scalar.copy(out=s2[:tb, :], in_=pst[:tb, :])

            # affine (mean + std*z) via full-rate f32r PE, sigmoid via ACT
            outb = outp.tile([P, BLK_T, NS], F32)
            for g in range(T // SIG_T):
                pa = paff_pool.tile([P, SIG_T * NS], F32)
                nc.tensor.matmul(
                    pa,
                    lhsT=s2[:tb, :],
                    rhs=z2_sb[:tb, g * SIG_T * NS : (g + 1) * SIG_T * NS],
                    start=True,
                    stop=True,
                )
                nc.scalar.activation(
                    out=outb[:, g * SIG_T : (g + 1) * SIG_T, :].rearrange(
                        "p t s -> p (t s)"
                    ),
                    in_=pa,
                    func=mybir.ActivationFunctionType.Sigmoid,
                )
            nc.sync.dma_start(
                out=outr[:, t0 : t0 + T, :], in_=outb[:, :T, :]
            )

    nc.finalize()
    return nc


def _trunc_f32r(x: np.ndarray) -> np.ndarray:
    return (
        np.ascontiguousarray(x, dtype=np.float32).view(np.uint32) & np.uint32(F32R_MASK)
    ).view(np.float32)


def _host_consts(w_mu: np.ndarray, w_log_var: np.ndarray, z: np.ndarray):
    elv = np.exp(w_log_var.astype(np.float32))
    wmu_rep = np.tile(w_mu.astype(np.float32)[None, :], (P, 1))
    elv_rep = np.tile(elv[None, :], (P, 1))
    z = np.asarray(z, dtype=np.float32)
    zh = _trunc_f32r(z)
    zl = _trunc_f32r(z - zh)
    ones = np.ones(NS, dtype=np.float32)
    z2big = np.zeros((KR * BLK_T, BLK_T * NS), dtype=np.float32)
    for j in range(BLK_T):
        c = slice(j * NS, (j + 1) * NS)
        z2big[KR * j + 0, c] = ones
        z2big[KR * j + 1, c] = ones
        z2big[KR * j + 2, c] = zh
        z2big[KR * j + 3, c] = zl
        z2big[KR * j + 4, c] = zh
    ident = np.eye(P, dtype=np.float32)
    return wmu_rep, elv_rep, z2big, ident


_PROGRAM_CACHE: dict[int, "bass.Bass"] = {}


def run(X, w_mu, w_log_var, z, trace=False):
    X = np.ascontiguousarray(X, dtype=np.float32)
    n = X.shape[0]
    assert n % N_CORES == 0
    rows = n // N_CORES
    if rows not in _PROGRAM_CACHE:
        _PROGRAM_CACHE[rows] = build_program(rows)
    nc = _PROGRAM_CACHE[rows]

    wmu_rep, elv_rep, z2big, ident = _host_consts(
        np.asarray(w_mu), np.asarray(w_log_var), np.asarray(z)
    )
    in_maps = [
        {
            "x": X[i * rows : (i + 1) * rows],
            "wmu": wmu_rep,
            "elv": elv_rep,
            "z2big": z2big,
            "ident": ident,
        }
        for i in range(N_CORES)
    ]
    res = run_bass_kernel_spmd(nc, in_maps, list(range(N_CORES)), trace=trace)
    outs = [res.results[i]["out"] for i in range(N_CORES)]
    full = np.concatenate(outs, axis=0)
    return full, res


def kernel(X, w_mu, w_log_var, z):
    full, _ = run(X, w_mu, w_log_var, z, trace=False)
    return full



# revision 2
# speedup vs baseline: 1.8358x; 1.8358x over previous
"""Trainium2 Bass kernel: sampled logistic-regression forward.

reference math (per data row i, sample s):
    mean_i = X[i] . w_mu
    var_i  = sum_d X[i,d]^2 * exp(w_log_var[d])
    out[i,s] = sigmoid( sqrt(var_i) * z[s] + mean_i )

Full shapes: X [500000, 64], w_mu [64], w_log_var [64], z [128]
Output: [500000, 128] fp32.

Sharding: data-parallel over 8 NeuronCores, 62500 rows each.

Layout: within a core, row r = p*500 + t maps to partition p, per-partition
tile t. Both the X load and the out store are then CONTIGUOUS per partition
(T*128 B in, T*256 B out per block) -- large DMA descriptors at HBM line
rate, vs the 256 B interleaved descriptors of a (t p) layout.

Precision: X is fed as fp16 (halves input traffic; max sigmoid error from
fp16 X/w/elv quantization measured at 3.3e-3 vs the 2e-2 gate), the output
is written as fp16 (adds <= 2.4e-4) and upcast on host.

Per-core pipeline, blocks of T<=32 tiles x [125 rows, 64]:
  - DMA in X block (fp16)
  - GPSIMD: X2 = X*X (f32), V = X2 * exp(lv) (bcast, in place)
  - DVE: A = X * w_mu (bcast, f32); reduce(A) -> mean; reduce(V) -> var
  - DVE: Newton rsqrt (bit-trick seed, 2 iters); std = var * y
  - stats split hi/lo (mh, ml, sh, sl) to f32r precision (mantissa
    AND-mask + exact remainder) into a 4-row-per-tile block [125, 4*T]
  - PE: transpose stats -> [4T, 125]; full-rate float32r matmuls vs a
    constant block-diagonal Z2BIG [128, 32*128]:
      arg = mh*1 + ml*1 + sh*zr + sl*zr = mean + std*z  (zr = RNE-f32r(z))
  - ACT: Sigmoid [125, 512] PSUM -> SBUF fp16 (4-tile batches)
  - DMA out block (fp16)

float32r matmuls stream 1 column/cycle (vs 1/4 for fp32); f32r keeps 12
mantissa bits, so mh+ml / sh+sl recover ~fp32 mean/std exactly and the
only affine error is the single f32r rounding of z (~2^-13 rel).
"""

from contextlib import ExitStack

import numpy as np

import concourse.bacc as bacc
import concourse.bass as bass
import concourse.tile as tile
from concourse import mybir
from concourse.bass_utils import run_bass_kernel_spmd

N_CORES = 8
D = 64
NS = 128
P = 125          # rows per tile (partition dim); 62500 = 125 * 500
BLK_T = 32       # tiles per block (4*32 = 128 = K of the affine matmul)
SIG_T = 4        # tiles per sigmoid ACT op (4*128 = 512 f32 = one PSUM bank)
KR = 4           # K-rows per tile: mh, ml, sh, sl

RSQRT_MAGIC = 0x5F3759DF
F32R_MASK = 0xFFFFF000   # keep 11 explicit mantissa bits (f32r-representable)
F16 = mybir.dt.float16
F32 = mybir.dt.float32
F32R = mybir.dt.float32r
U32 = mybir.dt.uint32


def build_program(rows: int):
    """Build the single-core Bass/Tile program for `rows` rows (SPMD across cores)."""
    assert rows % P == 0
    ntiles = rows // P
    assert ntiles % SIG_T == 0

    nc = bacc.Bacc(
        "TRN2",
        target_bir_lowering=False,
        debug=False,
        num_devices=N_CORES,
    )

    x = nc.dram_tensor("x", [rows, D], F16, kind="ExternalInput")
    wmu_d = nc.dram_tensor("wmu", [P, D], F16, kind="ExternalInput")
    elv_d = nc.dram_tensor("elv", [P, D], F32, kind="ExternalInput")
    z2big = nc.dram_tensor(
        "z2big", [KR * BLK_T, BLK_T * NS], F32R, kind="ExternalInput"
    )
    ident = nc.dram_tensor("ident", [P, P], F32, kind="ExternalInput")
    out = nc.dram_tensor("out", [rows, NS], F16, kind="ExternalOutput")

    # row r = p*ntiles + t: per-partition-contiguous in DRAM
    xr = x.rearrange("(p t) d -> p t d", p=P)        # [125, ntiles, 64]
    outr = out.rearrange("(p t) s -> p t s", p=P)    # [125, ntiles, 128]

    nblocks = (ntiles + BLK_T - 1) // BLK_T

    with tile.TileContext(nc) as tc, ExitStack() as ctx:
        singles = ctx.enter_context(tc.tile_pool(name="singles", bufs=1))
        xin = ctx.enter_context(tc.tile_pool(name="xin", bufs=4))
        sqp = ctx.enter_context(tc.tile_pool(name="sqp", bufs=3))
        amp = ctx.enter_context(tc.tile_pool(name="amp", bufs=3))
        statp = ctx.enter_context(tc.tile_pool(name="statp", bufs=3))
        smalls = ctx.enter_context(tc.tile_pool(name="smalls", bufs=4))
        s2p = ctx.enter_context(tc.tile_pool(name="s2p", bufs=3))
        outp = ctx.enter_context(tc.tile_pool(name="outp", bufs=3))
        pst_pool = ctx.enter_context(tc.tile_pool(name="pst", bufs=2, space="PSUM"))
        paff_pool = ctx.enter_context(tc.tile_pool(name="paff", bufs=4, space="PSUM"))

        # one-time loads; broadcast weights are landed on their consumer
        # engines via a copy (keeps per-instruction sync-wait fan-in low)
        wmu_stage = singles.tile([P, 1, D], F16)
        nc.sync.dma_start(out=wmu_stage, in_=wmu_d.rearrange("p (o d) -> p o d", d=D))
        wmu_sb = singles.tile([P, 1, D], F16)
        nc.vector.tensor_copy(wmu_sb, wmu_stage)
        elv_stage = singles.tile([P, 1, D], F32)
        nc.sync.dma_start(out=elv_stage, in_=elv_d.rearrange("p (o d) -> p o d", d=D))
        elv_sb = singles.tile([P, 1, D], F32)
        nc.gpsimd.tensor_copy(elv_sb, elv_stage)
        z2_sb = singles.tile([KR * BLK_T, BLK_T * NS], F32R)
        nc.sync.dma_start(out=z2_sb, in_=z2big[:, :])
        id_stage = singles.tile([P, P], F32)
        nc.sync.dma_start(out=id_stage, in_=ident[:, :])
        id_sb = singles.tile([P, P], F32)
        nc.vector.tensor_copy(id_sb, id_stage)
        magic_sb = singles.tile([P, BLK_T], U32)
        nc.vector.memset(magic_sb, RSQRT_MAGIC)
        one_sb = singles.tile([P, 1], U32)
        nc.vector.memset(one_sb, 1)
        mask_sb = singles.tile([P, 1], U32)
        nc.vector.memset(mask_sb, F32R_MASK)

        for b in range(nblocks):
            t0 = b * BLK_T
            T = min(BLK_T, ntiles - t0)
            tb = KR * T

            xt = xin.tile([P, BLK_T, D], F16)
            nc.sync.dma_start(out=xt[:, :T, :], in_=xr[:, t0 : t0 + T, :])

            # X^2 (f32) then V = X^2 * exp(lv), both on GPSIMD
            x2 = sqp.tile([P, BLK_T, D], F32)
            nc.gpsimd.tensor_mul(x2[:, :T, :], xt[:, :T, :], xt[:, :T, :])
            nc.gpsimd.tensor_mul(
                x2[:, :T, :], x2[:, :T, :], elv_sb.to_broadcast([P, T, D])
            )
            # A = X * w_mu (broadcast along tile dim) on DVE
            at = amp.tile([P, BLK_T, D], F32)
            nc.vector.tensor_mul(
                at[:, :T, :], xt[:, :T, :], wmu_sb.to_broadcast([P, T, D])
            )

            mean_t = smalls.tile([P, BLK_T], F32)
            nc.vector.tensor_reduce(
                out=mean_t[:, :T],
                in_=at[:, :T, :],
                axis=mybir.AxisListType.X,
                op=mybir.AluOpType.add,
            )
            var = smalls.tile([P, BLK_T], F32)
            nc.vector.tensor_reduce(
                out=var[:, :T],
                in_=x2[:, :T, :],
                axis=mybir.AxisListType.X,
                op=mybir.AluOpType.add,
            )

            # y = rsqrt(var) on DVE: seed 0x5f3759df - (bits >> 1), 2 NR iters
            vb = var[:, :T].bitcast(U32)
            yb = smalls.tile([P, BLK_T], U32)
            nc.vector.tensor_scalar(
                yb[:, :T], vb, one_sb[:, 0:1], None,
                op0=mybir.AluOpType.logical_shift_right,
            )
            nc.vector.scalar_tensor_tensor(
                out=yb[:, :T],
                in0=magic_sb[:, :T],
                scalar=0,
                in1=yb[:, :T],
                op0=mybir.AluOpType.bypass,
                op1=mybir.AluOpType.subtract,
            )
            y = yb.bitcast(F32)
            t2 = smalls.tile([P, BLK_T], F32)
            for _ in range(2):
                nc.vector.tensor_mul(t2[:, :T], y[:, :T], y[:, :T])
                nc.vector.tensor_mul(t2[:, :T], t2[:, :T], var[:, :T])
                nc.vector.tensor_scalar(
                    t2[:, :T], t2[:, :T], -0.5, 1.5,
                    op0=mybir.AluOpType.mult,
                    op1=mybir.AluOpType.add,
                )
                nc.vector.tensor_mul(y[:, :T], y[:, :T], t2[:, :T])
            std_t = smalls.tile([P, BLK_T], F32)
            nc.vector.tensor_mul(std_t[:, :T], var[:, :T], y[:, :T])

            # split mean/std into f32r-representable hi + exact f32 lo rows:
            # statblk rows per tile: [mh, ml, sh, sl] (lo rows re-round to
            # f32r inside the PE at ~2^-24 relative -- negligible)
            statblk = statp.tile([P, BLK_T, KR], F32)
            sb_u = statblk.bitcast(U32)
            nc.vector.tensor_scalar(
                sb_u[:, :T, 0], mean_t[:, :T].bitcast(U32), mask_sb[:, 0:1], None,
                op0=mybir.AluOpType.bitwise_and,
            )
            nc.vector.tensor_sub(
                statblk[:, :T, 1], mean_t[:, :T], statblk[:, :T, 0]
            )
            nc.vector.tensor_scalar(
                sb_u[:, :T, 2], std_t[:, :T].bitcast(U32), mask_sb[:, 0:1], None,
                op0=mybir.AluOpType.bitwise_and,
            )
            nc.vector.tensor_sub(
                statblk[:, :T, 3], std_t[:, :T], statblk[:, :T, 2]
            )

            # transpose stats block: [125, tb] -> [tb, 125] (PSUM), copy to SBUF
            pst = pst_pool.tile([KR * BLK_T, P], F32)
            nc.tensor.transpose(
                out=pst[:tb, :],
                in_=statblk.rearrange("p t k -> p (t k)")[:, :tb],
                identity=id_sb,
            )
            s2 = s2p.tile([KR * BLK_T, P], F32R)
            nc.scalar.copy(out=s2[:tb, :], in_=pst[:tb, :])

            # affine (mean + std*z) via full-rate f32r PE, sigmoid via ACT
            outb = outp.tile([P, BLK_T, NS], F16)
            for g in range(T // SIG_T):
                pa = paff_pool.tile([P, SIG_T * NS], F32)
                nc.tensor.matmul(
                    pa,
                    lhsT=s2[:tb, :],
                    rhs=z2_sb[:tb, g * SIG_T * NS : (g + 1) * SIG_T * NS],
                    start=True,
                    stop=True,
                )
                nc.scalar.activation(
                    out=outb[:, g * SIG_T : (g + 1) * SIG_T, :].rearrange(
                        "p t s -> p (t s)"
                    ),
                    in_=pa,
                    func=mybir.ActivationFunctionType.Sigmoid,
                )
            nc.sync.dma_start(
                out=outr[:, t0 : t0 + T, :], in_=outb[:, :T, :]
            )

    nc.finalize()
    return nc


def _round_f32r(x: np.ndarray) -> np.ndarray:
    """Round f32 values to the nearest f32r-representable (11 explicit
    mantissa bits) via add-half-then-truncate on the bit pattern."""
    b = np.ascontiguousarray(x, dtype=np.float32).view(np.uint32)
    return ((b + np.uint32(0x800)) & np.uint32(F32R_MASK)).view(np.float32)


def _host_consts(w_mu: np.ndarray, w_log_var: np.ndarray, z: np.ndarray):
    elv = np.exp(w_log_var.astype(np.float32))
    wmu_rep = np.tile(w_mu.astype(np.float16)[None, :], (P, 1))
    elv_rep = np.tile(elv[None, :], (P, 1))
    zr = _round_f32r(np.asarray(z, dtype=np.float32))
    ones = np.ones(NS, dtype=np.float32)
    z2big = np.zeros((KR * BLK_T, BLK_T * NS), dtype=np.float32)
    for j in range(BLK_T):
        c = slice(j * NS, (j + 1) * NS)
        z2big[KR * j + 0, c] = ones
        z2big[KR * j + 1, c] = ones
        z2big[KR * j + 2, c] = zr
        z2big[KR * j + 3, c] = zr
    ident = np.eye(P, dtype=np.float32)
    return wmu_rep, elv_rep, z2big, ident


_PROGRAM_CACHE: dict[int, "bass.Bass"] = {}


def run(X, w_mu, w_log_var, z, trace=False):
    X = np.ascontiguousarray(X).astype(np.float16)
    n = X.shape[0]
    assert n % N_CORES == 0
    rows = n // N_CORES
    if rows not in _PROGRAM_CACHE:
        _PROGRAM_CACHE[rows] = build_program(rows)
    nc = _PROGRAM_CACHE[rows]

    wmu_rep, elv_rep, z2big, ident = _host_consts(
        np.asarray(w_mu), np.asarray(w_log_var), np.asarray(z)
    )
    in_maps = [
        {
            "x": X[i * rows : (i + 1) * rows],
            "wmu": wmu_rep,
            "elv": elv_rep,
            "z2big": z2big,
            "ident": ident,
        }
        for i in range(N_CORES)
    ]
    res = run_bass_kernel_spmd(nc, in_maps, list(range(N_CORES)), trace=trace)
    outs = [res.results[i]["out"] for i in range(N_CORES)]
    full = np.concatenate(outs, axis=0).astype(np.float32)
    return full, res


def kernel(X, w_mu, w_log_var, z):
    full, _ = run(X, w_mu, w_log_var, z, trace=False)
    return full


# revision 5
# speedup vs baseline: 2.4392x; 1.3287x over previous
"""Trainium2 Bass kernel: sampled logistic-regression forward.

reference math (per data row i, sample s):
    mean_i = X[i] . w_mu
    var_i  = sum_d X[i,d]^2 * exp(w_log_var[d])
    out[i,s] = sigmoid( sqrt(var_i) * z[s] + mean_i )

Full shapes: X [500000, 64], w_mu [64], w_log_var [64], z [128]
Output: [500000, 128] fp32.

Sharding: data-parallel over 8 NeuronCores, 62500 rows each.

Layout: within a core, row r = p*500 + t maps to partition p, per-partition
tile t. Both the X load and the out store are then CONTIGUOUS per partition
(large DMA descriptors at HBM line rate).

Precision: X/w/elv are fed as fp16 (max sigmoid error from that
quantization measured at 3.3e-3 vs the 2e-2 gate); output is written fp16
(<= 2.4e-4 more) and upcast on host. The affine itself is numerically
~exact: mean/std are split hi/lo into fp16 pairs (mh+ml, sh+sl) and z into
zh+zl, and the PE accumulates their products in f32, recovering
mean + std*z to ~2^-22 relative.

Per-core pipeline, super-blocks of 4 blocks x [24 tiles x 125 rows]:
  per block:
  - DMA in X block (fp16)
  - DVE: X2 = X*X, A = X*w_mu (fp16, 2x rate)
  - GPSIMD: V = X2 * exp(lv) (bcast, in place)
  - DVE: reduce(A) -> mean, reduce(V) -> var (f32 out)
  per super-block (stats batched 4 blocks wide to amortize op overheads):
  - DVE: Newton rsqrt (bit-trick seed, 2 iters, plain tensor-tensor ops
    only -- the tensor_scalar/STT forms cost 0.9-2.9 us each on HW);
    std = var * y
  - DVE+GPSIMD: split mean/std into fp16-representable hi (mantissa AND
    mask) + exact f32 remainder: stat rows [mh, ml, sh, sh, sl]
  per block:
  - PE: transpose stats f32 [125, 5T] -> [5T, 125] PSUM; ACT copy-cast to
    fp16 s2
  - PE: fp16 matmuls vs constant block-diagonal Z2BIG [120, 24*128] with
    rows [1, 1, zh, zl, zh] per tile: arg = mean + std*z in f32 PSUM
    (fp16 streams 1 col/cycle at any PE p-state; products exact in f32)
  - ACT: Sigmoid [125, 1024] PSUM -> SBUF fp16 (8-tile batches)
  - DMA out block (fp16)
"""

from contextlib import ExitStack

import numpy as np

import concourse.bacc as bacc
import concourse.bass as bass
import concourse.tile as tile
from concourse import mybir
from concourse.bass_utils import run_bass_kernel_spmd

N_CORES = 8
D = 64
NS = 128
P = 125          # rows per tile (partition dim); 62500 = 125 * 500
BLK_T = 24       # tiles per block (5*24 = 120 = K of the affine matmul)
SIG_T = 4        # tiles per matmul (4*128 = 512 f32 = one PSUM bank)
PA_T = 8         # tiles per sigmoid ACT op (2 PSUM banks)
KR = 5           # K-rows per tile: mh, ml, sh, sh, sl
G = 4            # blocks per super-block (stats batching)

RSQRT_MAGIC = 0x5F3759DF
F16_MASK = 0xFFFFE000   # keep 10 explicit mantissa bits (fp16-representable)
F16 = mybir.dt.float16
F32 = mybir.dt.float32
U32 = mybir.dt.uint32


def build_program(rows: int):
    """Build the single-core Bass/Tile program for `rows` rows (SPMD across cores)."""
    assert rows % P == 0
    ntiles = rows // P

    nc = bacc.Bacc(
        "TRN2",
        target_bir_lowering=False,
        debug=False,
        num_devices=N_CORES,
    )

    x = nc.dram_tensor("x", [rows, D], F16, kind="ExternalInput")
    wmu_d = nc.dram_tensor("wmu", [P, D], F16, kind="ExternalInput")
    elv_d = nc.dram_tensor("elv", [P, D], F16, kind="ExternalInput")
    z2big = nc.dram_tensor(
        "z2big", [KR * BLK_T, BLK_T * NS], F16, kind="ExternalInput"
    )
    ident = nc.dram_tensor("ident", [P, P], F32, kind="ExternalInput")
    out = nc.dram_tensor("out", [rows, NS], F16, kind="ExternalOutput")

    # row r = p*ntiles + t: per-partition-contiguous in DRAM
    xr = x.rearrange("(p t) d -> p t d", p=P)        # [125, ntiles, 64]
    outr = out.rearrange("(p t) s -> p t s", p=P)    # [125, ntiles, 128]

    nblocks = (ntiles + BLK_T - 1) // BLK_T
    nsupers = (nblocks + G - 1) // G

    with tile.TileContext(nc) as tc, ExitStack() as ctx:
        singles = ctx.enter_context(tc.tile_pool(name="singles", bufs=1))
        xin = ctx.enter_context(tc.tile_pool(name="xin", bufs=8))
        sqp = ctx.enter_context(tc.tile_pool(name="sqp", bufs=6))
        amp = ctx.enter_context(tc.tile_pool(name="amp", bufs=6))
        mvp = ctx.enter_context(tc.tile_pool(name="mvp", bufs=2))
        statp = ctx.enter_context(tc.tile_pool(name="statp", bufs=2))
        smalls = ctx.enter_context(tc.tile_pool(name="smalls", bufs=2))
        s2p = ctx.enter_context(tc.tile_pool(name="s2p", bufs=3))
        outp = ctx.enter_context(tc.tile_pool(name="outp", bufs=4))
        pst_pool = ctx.enter_context(tc.tile_pool(name="pst", bufs=2, space="PSUM"))
        paff_pool = ctx.enter_context(tc.tile_pool(name="paff", bufs=3, space="PSUM"))

        # one-time loads; broadcast weights are landed on their consumer
        # engines via a copy (keeps per-instruction sync-wait fan-in low)
        wmu_stage = singles.tile([P, 1, D], F16)
        nc.sync.dma_start(out=wmu_stage, in_=wmu_d.rearrange("p (o d) -> p o d", d=D))
        wmu_sb = singles.tile([P, 1, D], F16)
        nc.vector.tensor_copy(wmu_sb, wmu_stage)
        elv_stage = singles.tile([P, 1, D], F16)
        nc.sync.dma_start(out=elv_stage, in_=elv_d.rearrange("p (o d) -> p o d", d=D))
        elv_sb = singles.tile([P, 1, D], F16)
        nc.gpsimd.tensor_copy(elv_sb, elv_stage)
        z2_sb = singles.tile([KR * BLK_T, BLK_T * NS], F16)
        nc.sync.dma_start(out=z2_sb, in_=z2big[:, :])
        id_stage = singles.tile([P, P], F32)
        nc.sync.dma_start(out=id_stage, in_=ident[:, :])
        id_sb = singles.tile([P, P], F32)
        nc.vector.tensor_copy(id_sb, id_stage)
        magic_sb = singles.tile([P, 1], U32)
        nc.vector.memset(magic_sb, RSQRT_MAGIC)
        one_sb = singles.tile([P, 1], U32)
        nc.vector.memset(one_sb, 1)
        mask_sb = singles.tile([P, 1], U32)
        nc.vector.memset(mask_sb, F16_MASK)
        half_sb = singles.tile([P, 1], F32)
        nc.vector.memset(half_sb, 0.5)
        c15_sb = singles.tile([P, 1], F32)
        nc.vector.memset(c15_sb, 1.5)

        def b1(ap, shape):
            return ap.to_broadcast(shape)

        for s in range(nsupers):
            b0 = s * G
            Gs = min(G, nblocks - b0)
            # tiles covered by this super; per-block T (only last block short)
            Ts = [min(BLK_T, ntiles - (b0 + bi) * BLK_T) for bi in range(Gs)]
            uniform = all(t == BLK_T for t in Ts)

            mv = mvp.tile([P, G, 2, BLK_T], F32)

            for bi in range(Gs):
                b = b0 + bi
                t0 = b * BLK_T
                T = Ts[bi]

                xt = xin.tile([P, BLK_T, D], F16)
                nc.sync.dma_start(out=xt[:, :T, :], in_=xr[:, t0 : t0 + T, :])

                # X^2 and A = X*w_mu on DVE (fp16); V = X^2 * elv on GPSIMD
                x2 = sqp.tile([P, BLK_T, D], F16)
                nc.vector.tensor_mul(x2[:, :T, :], xt[:, :T, :], xt[:, :T, :])
                at = amp.tile([P, BLK_T, D], F16)
                nc.vector.tensor_mul(
                    at[:, :T, :], xt[:, :T, :], b1(wmu_sb, [P, T, D])
                )
                nc.gpsimd.tensor_mul(
                    x2[:, :T, :], x2[:, :T, :], b1(elv_sb, [P, T, D])
                )
                nc.vector.tensor_reduce(
                    out=mv[:, bi, 0, :T],
                    in_=at[:, :T, :],
                    axis=mybir.AxisListType.X,
                    op=mybir.AluOpType.add,
                )
                nc.vector.tensor_reduce(
                    out=mv[:, bi, 1, :T],
                    in_=x2[:, :T, :],
                    axis=mybir.AxisListType.X,
                    op=mybir.AluOpType.add,
                )

            # ---- batched stats for the whole super-block ----
            # views [P, Gs, Tb]; for the (only) short tail super Gs == 1
            Tb = Ts[0] if uniform else max(Ts)
            mean = mv[:, :Gs, 0, :Tb]
            var = mv[:, :Gs, 1, :Tb]
            shp = [P, Gs, Tb]

            # y = rsqrt(var): seed 0x5f3759df - (bits >> 1), then 2 NR iters
            # written as plain tensor_tensor ops (sign flip of the
            # (h*y^2 - 1.5) form cancels over the two iterations)
            yb_t = smalls.tile([P, G, BLK_T], U32)
            yb = yb_t[:, :Gs, :Tb]
            nc.vector.tensor_tensor(
                yb, var.bitcast(U32), b1(one_sb[:, 0:1], shp),
                op=mybir.AluOpType.logical_shift_right,
            )
            nc.vector.tensor_tensor(
                yb, b1(magic_sb[:, 0:1], shp), yb, op=mybir.AluOpType.subtract
            )
            y = yb.bitcast(F32)
            hv_t = smalls.tile([P, G, BLK_T], F32)
            hv = hv_t[:, :Gs, :Tb]
            nc.vector.tensor_tensor(
                hv, var, b1(half_sb[:, 0:1], shp), op=mybir.AluOpType.mult
            )
            t2_t = smalls.tile([P, G, BLK_T], F32)
            t2 = t2_t[:, :Gs, :Tb]
            for _ in range(2):
                nc.vector.tensor_mul(t2, y, y)
                nc.vector.tensor_mul(t2, t2, hv)
                nc.vector.tensor_tensor(
                    t2, t2, b1(c15_sb[:, 0:1], shp), op=mybir.AluOpType.subtract
                )
                nc.vector.tensor_mul(y, y, t2)
            std_t = smalls.tile([P, G, BLK_T], F32)
            std = std_t[:, :Gs, :Tb]
            nc.vector.tensor_mul(std, var, y)

            # split mean/std into fp16-representable hi + exact f32 lo:
            # stat rows per tile: [mh, ml, sh, sh, sl]
            stat = statp.tile([P, G, BLK_T, KR], F32)
            sv = stat[:, :Gs, :Tb, :]
            su = sv.bitcast(U32)
            maskb = b1(mask_sb[:, 0:1], shp)
            nc.vector.tensor_tensor(
                su[:, :, :, 0], mean.bitcast(U32), maskb,
                op=mybir.AluOpType.bitwise_and,
            )
            nc.gpsimd.tensor_sub(sv[:, :, :, 1], mean, sv[:, :, :, 0])
            nc.vector.tensor_tensor(
                su[:, :, :, 2], std.bitcast(U32), maskb,
                op=mybir.AluOpType.bitwise_and,
            )
            nc.gpsimd.tensor_copy(sv[:, :, :, 3], sv[:, :, :, 2])
            nc.gpsimd.tensor_sub(sv[:, :, :, 4], std, sv[:, :, :, 2])

            # ---- affine + sigmoid + store per block ----
            for bi in range(Gs):
                b = b0 + bi
                t0 = b * BLK_T
                T = Ts[bi]
                tb = KR * T

                pst = pst_pool.tile([KR * BLK_T, P], F32)
                nc.tensor.transpose(
                    out=pst[:tb, :],
                    in_=stat[:, bi].rearrange("p t k -> p (t k)")[:, :tb],
                    identity=id_sb,
                )
                s2 = s2p.tile([KR * BLK_T, P], F16)
                nc.scalar.copy(out=s2[:tb, :], in_=pst[:tb, :])

                outb = outp.tile([P, BLK_T, NS], F16)
                for c in range(0, T, PA_T):
                    ct = min(PA_T, T - c)
                    pa = paff_pool.tile([P, PA_T * NS], F32)
                    for g0 in range(0, ct, SIG_T):
                        j0 = c + g0
                        nc.tensor.matmul(
                            pa[:, g0 * NS : (g0 + SIG_T) * NS],
                            lhsT=s2[:tb, :],
                            rhs=z2_sb[:tb, j0 * NS : (j0 + SIG_T) * NS],
                            start=True,
                            stop=True,
                        )
                    nc.scalar.activation(
                        out=outb[:, c : c + ct, :].rearrange("p t s -> p (t s)"),
                        in_=pa[:, : ct * NS],
                        func=mybir.ActivationFunctionType.Sigmoid,
                    )
                nc.sync.dma_start(
                    out=outr[:, t0 : t0 + T, :], in_=outb[:, :T, :]
                )

    nc.finalize()
    return nc


def _host_consts(w_mu: np.ndarray, w_log_var: np.ndarray, z: np.ndarray):
    elv = np.exp(w_log_var.astype(np.float32))
    wmu_rep = np.tile(w_mu.astype(np.float16)[None, :], (P, 1))
    elv_rep = np.tile(elv.astype(np.float16)[None, :], (P, 1))
    z = np.asarray(z, dtype=np.float32)
    zh = z.astype(np.float16)
    zl = (z - zh.astype(np.float32)).astype(np.float16)
    ones = np.ones(NS, dtype=np.float16)
    z2big = np.zeros((KR * BLK_T, BLK_T * NS), dtype=np.float16)
    for j in range(BLK_T):
        c = slice(j * NS, (j + 1) * NS)
        z2big[KR * j + 0, c] = ones
        z2big[KR * j + 1, c] = ones
        z2big[KR * j + 2, c] = zh
        z2big[KR * j + 3, c] = zl
        z2big[KR * j + 4, c] = zh
    ident = np.eye(P, dtype=np.float32)
    return wmu_rep, elv_rep, z2big, ident


_PROGRAM_CACHE: dict[int, "bass.Bass"] = {}


def run(X, w_mu, w_log_var, z, trace=False):
    X = np.ascontiguousarray(X).astype(np.float16)
    n = X.shape[0]
    assert n % N_CORES == 0
    rows = n // N_CORES
    if rows not in _PROGRAM_CACHE:
        _PROGRAM_CACHE[rows] = build_program(rows)
    nc = _PROGRAM_CACHE[rows]

    wmu_rep, elv_rep, z2big, ident = _host_consts(
        np.asarray(w_mu), np.asarray(w_log_var), np.asarray(z)
    )
    in_maps = [
        {
            "x": X[i * rows : (i + 1) * rows],
            "wmu": wmu_rep,
            "elv": elv_rep,
            "z2big": z2big,
            "ident": ident,
        }
        for i in range(N_CORES)
    ]
    res = run_bass_kernel_spmd(nc, in_maps, list(range(N_CORES)), trace=trace)
    outs = [res.results[i]["out"] for i in range(N_CORES)]
    full = np.concatenate(outs, axis=0).astype(np.float32)
    return full, res


def kernel(X, w_mu, w_log_var, z):
    full, _ = run(X, w_mu, w_log_var, z, trace=False)
    return full
